# revision 8
# baseline (speedup 1.0000x reference)
"""KAN layer (B=8192, IN_F=OUT_F=1024, GRID=5) on 8 Trainium2 cores.

Math: Y[b,o] = W0[o]*silu(x) + spline_o(clip(x,-1,1)) + b[o], x = X[b,o]
(idx_in = arange(O) % IN_F is the identity here since O == IN_F).

The degree-1 B-spline on the uniform 5-knot grid over [-1,1] is rewritten in
the clipped-ramp (segment) basis: with knots s_j in {-1,-0.5,0,0.5} and
t_j = clip(x, s_j, s_j+0.5),  spline(clip(x)) = A'' + sum_j m_j * t_j,
m_j = 2*(c_{j+1}-c_j).  Each t_j is ONE tensor_scalar(min,max) from the raw
(unclipped) x.  Folding W1 and b gives Y^T = W0*silu(x) + sum_j wm_j*t_j + A'.

Layout: edges on SBUF partitions (X pre-transposed AND cast to fp16 on host),
batch on the free dim, data-parallel over batch across the 8 cores.  Per
128-edge block the weighted sum of the 5 fp16 feature maps (t0..t3, silu)
runs on TensorE as diagonal-stationary matmuls accumulating in PSUM (fp16 is
the fastest PE feed on this stack: fp8 DoubleRow measured at the same
cycles/col, so it only adds DVE cost).  Diagonal stationaries are built on
device by ONE broadcast tensor_tensor per feature during the DMA-wait
preamble (DVE is idle there).  ScalarE computes silu; VectorE computes the
four t-features at the 4x DVE rate (fp16-in/fp16-out, contiguous; GpSimd
must NOT run concurrent elementwise ops — shared SBUF ports halve DVE).
PSUM evacuation (+ per-edge bias A', fp32->fp16) splits ScalarE/VectorE
(GPSIMD cannot read PSUM).  Output is stored fp16, widened on host.
DMA: x on Sync, constants on the Scalar queue, stores on GpSimd SWDGE.
"""
import sys

for _p in ("/root/.axon_site", "/root/.axon_site/_ro/trn_rl_repo", "/root/.axon_site/_ro/pypackages"):
    if _p not in sys.path:
        sys.path.append(_p)

import numpy as np

import concourse.bacc as bacc
import concourse.tile as tile
from concourse import mybir
from concourse.bass_utils import run_bass_kernel_spmd

B, IN_F, OUT_F, GRID = 8192, 1024, 1024, 5
N_CORES = 8
B_SHARD = B // N_CORES          # 1024 batch rows per core
EB = OUT_F // 128               # 8 edge blocks
NF = 5                          # features: t0..t3, silu
CHUNK = 512                     # one PSUM bank of fp32

DVE_EVAC = (2, 4, 5)            # blocks evacuated by VectorE (rest ScalarE)
X_CHUNKS = ((0, 1), (1, 2), (2, 4), (4, 6), (6, 8))   # input DMA block spans
TBOUNDS = ((-0.5, -1.0), (0.0, -0.5), (0.5, 0.0), (1.0, 0.5))  # (hi, lo)

_nc_cache = None


def _build():
    f32 = mybir.dt.float32
    f16 = mybir.dt.float16
    AF = mybir.ActivationFunctionType
    OP = mybir.AluOpType
    nc = bacc.Bacc("TRN2", target_bir_lowering=False, debug=False)
    xt = nc.dram_tensor("xt", [OUT_F, B_SHARD], f16, kind="ExternalInput").ap()
    # cpack: [0:128] fp16 identity, [128:168] fp16 weights (feat-major, 5x8),
    # packed on 128 fp32 columns; apr separate fp32 [128, EB]
    cw = nc.dram_tensor("cw", [128, 168], f16, kind="ExternalInput").ap()
    apr = nc.dram_tensor("apr", [128, EB], f32, kind="ExternalInput").ap()
    yt = nc.dram_tensor("yt", [OUT_F, B_SHARD], f16, kind="ExternalOutput").ap()

    xt3 = xt.rearrange("(n p) d -> p n d", p=128)   # [128, EB, B_SHARD]
    yt3 = yt.rearrange("(n p) d -> p n d", p=128)

    with tile.TileContext(nc) as tc:
        with tc.tile_pool(name="const", bufs=1) as const_pool, \
             tc.tile_pool(name="xin", bufs=3) as xin_pool, \
             tc.tile_pool(name="feat", bufs=3) as feat_pool, \
             tc.tile_pool(name="silup", bufs=3) as silu_pool, \
             tc.tile_pool(name="yout", bufs=3) as yout_pool, \
             tc.tile_pool(name="ps", bufs=3, space="PSUM") as psum_pool, \
             tc.tile_pool(name="pswarm", bufs=1, space="PSUM") as warm_pool:
            cw_t = const_pool.tile([128, 168], f16)
            apr_t = const_pool.tile([128, EB], f32)
            nc.scalar.dma_start(cw_t[:], cw[:, :])
            nc.scalar.dma_start(apr_t[:], apr[:, :])
            ident = cw_t[:, 0:128]
            wv = cw_t[:, 128:168].rearrange("p (j e) -> p j e", j=NF)

            # HAM warm-up: dummy matmuls on scratch SBUF so the PE clock gate
            # opens before the first real matmul arrives
            scratch = const_pool.tile([128, CHUNK], f16)
            nc.vector.memset(scratch[:], 0.0)
            ps_warm = warm_pool.tile([128, CHUNK], f32, tag="pswarm", name="pswarm")
            for _ in range(9):
                nc.tensor.matmul(ps_warm[:], scratch[:, 0:128], scratch[:],
                                 start=True, stop=True, skip_group_check=True)

            # diagonal stationaries, one broadcast multiply per feature
            # (runs on DVE during the x-DMA wait; j needed earliest first)
            d16 = const_pool.tile([128, NF, EB, 128], f16)
            for j in range(NF):
                nc.vector.tensor_tensor(
                    d16[:, j, :, :],
                    ident[:, None, :].broadcast_to([128, EB, 128]),
                    wv[:, j, :, None].broadcast_to([128, EB, 128]),
                    op=OP.mult)

            for b0, b1 in X_CHUNKS:
                nb = b1 - b0
                x_t = xin_pool.tile([128, nb, B_SHARD], f16, tag=f"x{nb}",
                                    name=f"x_{b0}")
                nc.sync.dma_start(x_t[:], xt3[:, b0:b1, :])
                for h in range(nb):
                    e = b0 + h
                    xv = x_t[:, h, :]
                    feats = feat_pool.tile([128, 4, B_SHARD], f16, tag="tf",
                                           name=f"tf_{e}")
                    for j, (hi, lo) in enumerate(TBOUNDS):
                        nc.vector.tensor_scalar(feats[:, j, :], xv, hi, lo,
                                                OP.min, OP.max)
                    silu_t = silu_pool.tile([128, B_SHARD], f16, tag="sl",
                                            name=f"sl_{e}")
                    nc.scalar.activation(silu_t[:], xv, AF.Silu)

                    ps = psum_pool.tile([128, B_SHARD], f32, tag="ps",
                                        name=f"ps_{e}")
                    for j in range(4):
                        for t in range(2):
                            cs = slice(t * CHUNK, (t + 1) * CHUNK)
                            nc.tensor.matmul(ps[:, cs], d16[:, j, e, :],
                                             feats[:, j, cs], start=(j == 0),
                                             stop=False, skip_group_check=True)
                    for t in range(2):
                        cs = slice(t * CHUNK, (t + 1) * CHUNK)
                        nc.tensor.matmul(ps[:, cs], d16[:, 4, e, :],
                                         silu_t[:, cs], start=False, stop=True,
                                         skip_group_check=True)

                    yo = yout_pool.tile([128, B_SHARD], f16, tag="yo",
                                        name=f"yo_{e}")
                    if e in DVE_EVAC:
                        nc.vector.tensor_scalar_add(yo[:], ps[:],
                                                    apr_t[:, e:e + 1])
                        nc.gpsimd.dma_start(yt3[:, e:e + 1, :], yo[:, None, :])
                    elif e < EB - 1:
                        nc.scalar.activation(yo[:], ps[:], AF.Identity,
                                             bias=apr_t[:, e:e + 1], scale=1.0)
                        nc.gpsimd.dma_start(yt3[:, e:e + 1, :], yo[:, None, :])
                    else:
                        # last block: per-chunk evac + store for a short tail
                        for t in range(2):
                            cs = slice(t * CHUNK, (t + 1) * CHUNK)
                            nc.scalar.activation(yo[:, cs], ps[:, cs],
                                                 AF.Identity,
                                                 bias=apr_t[:, e:e + 1],
                                                 scale=1.0)
                            nc.gpsimd.dma_start(yt3[:, e:e + 1, cs],
                                                yo[:, None, cs])
    nc.compile()
    return nc


def _host_prep(X, coeffs, W, b):
    c = coeffs.astype(np.float64)
    Wd = W.astype(np.float64)
    bd = b.astype(np.float64)
    m = 2.0 * (c[:, 1:] - c[:, :-1])             # [O, 4] slopes per unit x
    w1 = Wd[:, 1]
    wm = w1[:, None] * m                          # [O, 4] per-edge t weights
    s = np.array([-1.0, -0.5, 0.0, 0.5])
    aprime = bd + w1 * c[:, 0] - (wm * s[None, :]).sum(1)

    wvec = np.concatenate([wm, Wd[:, 0:1]], axis=1)        # [O, 5]
    wT = wvec.reshape(EB, 128, NF).transpose(1, 2, 0)       # [128, NF, EB]
    cw = np.zeros((128, 168), dtype=np.float16)
    cw[:, 0:128] = np.eye(128, dtype=np.float16)
    cw[:, 128:168] = wT.reshape(128, NF * EB).astype(np.float16)
    apr = aprime.reshape(EB, 128).transpose(1, 0).astype(np.float32)
    return cw, apr


def make_in_maps(X, coeffs, W, b):
    cw, apr = _host_prep(X, coeffs, W, b)
    x16 = X.astype(np.float16)
    in_maps = []
    for c in range(N_CORES):
        xt_shard = np.ascontiguousarray(x16[c * B_SHARD:(c + 1) * B_SHARD, :].T)
        in_maps.append({"xt": xt_shard, "cw": cw, "apr": apr})
    return in_maps


def kernel(X, coeffs, W, b):
    global _nc_cache
    if _nc_cache is None:
        _nc_cache = _build()
    nc = _nc_cache

    in_maps = make_in_maps(X, coeffs, W, b)
    res = run_bass_kernel_spmd(nc, in_maps, core_ids=list(range(N_CORES)))
    Y = np.empty((B, OUT_F), dtype=np.float32)
    for c in range(N_CORES):
        Y[c * B_SHARD:(c + 1) * B_SHARD, :] = res.results[c]["yt"].T.astype(np.float32)
    return Y


# revision 9
# speedup vs baseline: 1.0650x; 1.0650x over previous
"""KAN layer (B=8192, IN_F=OUT_F=1024, GRID=5) on 8 Trainium2 cores.

Math: Y[b,o] = W0[o]*silu(x) + spline_o(clip(x,-1,1)) + b[o], x = X[b,o]
(idx_in = arange(O) % IN_F is the identity here since O == IN_F).

The degree-1 B-spline on the uniform 5-knot grid over [-1,1] is rewritten in
the clipped-ramp (segment) basis: with knots s_j in {-1,-0.5,0,0.5} and
t_j = clip(x, s_j, s_j+0.5),  spline(clip(x)) = A'' + sum_j m_j * t_j,
m_j = 2*(c_{j+1}-c_j).  Each t_j is ONE tensor_scalar(min,max) from the raw
(unclipped) x.  Folding W1 and b gives Y^T = W0*silu(x) + sum_j wm_j*t_j + A'.

Layout: edges on SBUF partitions (X pre-transposed AND cast to fp16 on host),
batch on the free dim, data-parallel over batch across the 8 cores.  Per
128-edge block the weighted sum of the 5 fp16 feature maps (t0..t3, silu)
runs on TensorE as diagonal-stationary matmuls accumulating in PSUM (fp16 is
the fastest PE feed on this stack: fp8 DoubleRow measured at the same
cycles/col, so it only adds DVE cost).  Diagonal stationaries are built on
HOST and shipped in two DMAs on the Scalar queue (blocks 0-1 land before the
first matmul needs them; DMA bandwidth has large slack vs the PE wall).
ScalarE computes silu; VectorE computes the four t-features at the 4x DVE
rate (fp16-in/fp16-out contiguous; GpSimd must NOT run concurrent
elementwise ops - shared SBUF ports halve DVE).  PSUM evacuation (+ per-edge
bias A', fp32->fp16) splits ScalarE/VectorE one block behind the matmuls
(GPSIMD cannot read PSUM).  Output is stored fp16, widened on host.
DMA: x loads on Sync, constants on Scalar queue, stores also on Sync (the
GpSimd SWDGE drain costs ~2.6us at the tail; Sync's drain is ~0.4us).
"""
import sys

for _p in ("/root/.axon_site", "/root/.axon_site/_ro/trn_rl_repo", "/root/.axon_site/_ro/pypackages"):
    if _p not in sys.path:
        sys.path.append(_p)

import numpy as np

import concourse.bacc as bacc
import concourse.tile as tile
from concourse import mybir
from concourse.bass_utils import run_bass_kernel_spmd

B, IN_F, OUT_F, GRID = 8192, 1024, 1024, 5
N_CORES = 8
B_SHARD = B // N_CORES          # 1024 batch rows per core
EB = OUT_F // 128               # 8 edge blocks
NF = 5                          # features: t0..t3, silu
CHUNK = 512                     # one PSUM bank of fp32

DVE_EVAC = (2, 4)               # blocks evacuated by VectorE (rest ScalarE)
X_CHUNKS = ((0, 1), (1, 2), (2, 4), (4, 6), (6, 8))   # input DMA block spans
TBOUNDS = ((-0.5, -1.0), (0.0, -0.5), (0.5, 0.0), (1.0, 0.5))  # (hi, lo)

_nc_cache = None


def _build():
    f32 = mybir.dt.float32
    f16 = mybir.dt.float16
    AF = mybir.ActivationFunctionType
    OP = mybir.AluOpType
    nc = bacc.Bacc("TRN2", target_bir_lowering=False, debug=False)
    xt = nc.dram_tensor("xt", [OUT_F, B_SHARD], f16, kind="ExternalInput").ap()
    d16a = nc.dram_tensor("d16a", [128, NF, 2, 128], f16, kind="ExternalInput").ap()
    d16b = nc.dram_tensor("d16b", [128, NF, EB - 2, 128], f16,
                          kind="ExternalInput").ap()
    apr = nc.dram_tensor("apr", [128, EB], f32, kind="ExternalInput").ap()
    yt = nc.dram_tensor("yt", [OUT_F, B_SHARD], f16, kind="ExternalOutput").ap()

    xt3 = xt.rearrange("(n p) d -> p n d", p=128)   # [128, EB, B_SHARD]
    yt3 = yt.rearrange("(n p) d -> p n d", p=128)

    with tile.TileContext(nc) as tc:
        with tc.tile_pool(name="const", bufs=1) as const_pool, \
             tc.tile_pool(name="xin", bufs=3) as xin_pool, \
             tc.tile_pool(name="feat", bufs=3) as feat_pool, \
             tc.tile_pool(name="silup", bufs=3) as silu_pool, \
             tc.tile_pool(name="yout", bufs=3) as yout_pool, \
             tc.tile_pool(name="ps", bufs=3, space="PSUM") as psum_pool, \
             tc.tile_pool(name="pswarm", bufs=1, space="PSUM") as warm_pool:
            apr_t = const_pool.tile([128, EB], f32)
            d16 = const_pool.tile([128, NF, EB, 128], f16)
            nc.scalar.dma_start(apr_t[:], apr[:, :])
            nc.scalar.dma_start(d16[:, :, 0:2, :], d16a[:, :, :, :])
            nc.scalar.dma_start(d16[:, :, 2:EB, :], d16b[:, :, :, :])

            # HAM warm-up: dummy matmuls (scratch zeroed on GpSimd - it is
            # idle in the preamble and must not contend with DVE later) so
            # the PE clock ramps to full speed before real matmuls arrive
            scratch = const_pool.tile([128, CHUNK], f16)
            nc.gpsimd.memset(scratch[:], 0.0)
            ps_warm = warm_pool.tile([128, CHUNK], f32, tag="pswarm", name="pswarm")
            for _ in range(9):
                nc.tensor.matmul(ps_warm[:], scratch[:, 0:128], scratch[:],
                                 start=True, stop=True, skip_group_check=True)

            def emit_evac(e, ps, yo):
                if e in DVE_EVAC:
                    nc.vector.tensor_scalar_add(yo[:], ps[:], apr_t[:, e:e + 1])
                    nc.sync.dma_start(yt3[:, e:e + 1, :], yo[:, None, :])
                elif e < EB - 1:
                    nc.scalar.activation(yo[:], ps[:], AF.Identity,
                                         bias=apr_t[:, e:e + 1], scale=1.0)
                    nc.sync.dma_start(yt3[:, e:e + 1, :], yo[:, None, :])
                else:
                    # last block: halves on both engines for a short tail
                    cs0, cs1 = slice(0, CHUNK), slice(CHUNK, B_SHARD)
                    nc.scalar.activation(yo[:, cs0], ps[:, cs0], AF.Identity,
                                         bias=apr_t[:, e:e + 1], scale=1.0)
                    nc.sync.dma_start(yt3[:, e:e + 1, cs0], yo[:, None, cs0])
                    nc.vector.tensor_scalar_add(yo[:, cs1], ps[:, cs1],
                                                apr_t[:, e:e + 1])
                    nc.sync.dma_start(yt3[:, e:e + 1, cs1], yo[:, None, cs1])

            pending = None   # (e, ps, yo) evac'd one block behind the matmuls
            for b0, b1 in X_CHUNKS:
                nb = b1 - b0
                x_t = xin_pool.tile([128, nb, B_SHARD], f16, tag=f"x{nb}",
                                    name=f"x_{b0}")
                nc.sync.dma_start(x_t[:], xt3[:, b0:b1, :])
                for h in range(nb):
                    e = b0 + h
                    xv = x_t[:, h, :]
                    feats = feat_pool.tile([128, 4, B_SHARD], f16, tag="tf",
                                           name=f"tf_{e}")
                    for j, (hi, lo) in enumerate(TBOUNDS):
                        nc.vector.tensor_scalar(feats[:, j, :], xv, hi, lo,
                                                OP.min, OP.max)
                    silu_t = silu_pool.tile([128, B_SHARD], f16, tag="sl",
                                            name=f"sl_{e}")
                    nc.scalar.activation(silu_t[:], xv, AF.Silu)

                    ps = psum_pool.tile([128, B_SHARD], f32, tag="ps",
                                        name=f"ps_{e}")
                    for j in range(4):
                        for t in range(2):
                            cs = slice(t * CHUNK, (t + 1) * CHUNK)
                            nc.tensor.matmul(ps[:, cs], d16[:, j, e, :],
                                             feats[:, j, cs], start=(j == 0),
                                             stop=False, skip_group_check=True)
                    for t in range(2):
                        cs = slice(t * CHUNK, (t + 1) * CHUNK)
                        nc.tensor.matmul(ps[:, cs], d16[:, 4, e, :],
                                         silu_t[:, cs], start=False, stop=True,
                                         skip_group_check=True)

                    yo = yout_pool.tile([128, B_SHARD], f16, tag="yo",
                                        name=f"yo_{e}")
                    if pending is not None:
                        emit_evac(*pending)
                    pending = (e, ps, yo)
            emit_evac(*pending)
    nc.compile()
    return nc


def _host_prep(X, coeffs, W, b):
    c = coeffs.astype(np.float64)
    Wd = W.astype(np.float64)
    bd = b.astype(np.float64)
    m = 2.0 * (c[:, 1:] - c[:, :-1])             # [O, 4] slopes per unit x
    w1 = Wd[:, 1]
    wm = w1[:, None] * m                          # [O, 4] per-edge t weights
    s = np.array([-1.0, -0.5, 0.0, 0.5])
    aprime = bd + w1 * c[:, 0] - (wm * s[None, :]).sum(1)

    wvec = np.concatenate([wm, Wd[:, 0:1]], axis=1)        # [O, 5]
    wT = wvec.reshape(EB, 128, NF).transpose(1, 2, 0)       # [128, NF, EB]
    eye = np.eye(128, dtype=np.float64)
    d16 = (wT[:, :, :, None] * eye[:, None, None, :]).astype(np.float16)
    apr = aprime.reshape(EB, 128).transpose(1, 0).astype(np.float32)
    return d16, apr


def make_in_maps(X, coeffs, W, b):
    d16, apr = _host_prep(X, coeffs, W, b)
    d16a = np.ascontiguousarray(d16[:, :, 0:2, :])
    d16b = np.ascontiguousarray(d16[:, :, 2:EB, :])
    x16 = X.astype(np.float16)
    in_maps = []
    for c in range(N_CORES):
        xt_shard = np.ascontiguousarray(x16[c * B_SHARD:(c + 1) * B_SHARD, :].T)
        in_maps.append({"xt": xt_shard, "d16a": d16a, "d16b": d16b, "apr": apr})
    return in_maps


def kernel(X, coeffs, W, b):
    global _nc_cache
    if _nc_cache is None:
        _nc_cache = _build()
    nc = _nc_cache

    in_maps = make_in_maps(X, coeffs, W, b)
    res = run_bass_kernel_spmd(nc, in_maps, core_ids=list(range(N_CORES)))
    Y = np.empty((B, OUT_F), dtype=np.float32)
    for c in range(N_CORES):
        Y[c * B_SHARD:(c + 1) * B_SHARD, :] = res.results[c]["yt"].T.astype(np.float32)
    return Y
